# revision 19
# baseline (speedup 1.0000x reference)
"""Trainium2 Bass kernel for nn_AttentionNet (3-layer encoder + 1-layer masked
decoder + pointer log-softmax head), data-parallel over batch on 8 NeuronCores.

Layout strategy (per core, 2 batch items processed sequentially):
  - Activations live transposed in SBUF: [EMB=128 partitions, T free].
  - LayerNorm stats via ones-matmul partition reductions; rstd = exp(-0.5*ln(var+eps))
    so the whole kernel needs only the natural_log_exp_and_others ACT table set.
  - Attention computed per head in U^T = K Q^T form ([T_k part, T_q free]):
    softmax denominator comes from an extra ones-column in the V blocks, so no
    max-subtraction / no transposes of A are needed.
  - Q/K are produced in "even/odd" packed layouts (4 heads at partition bases
    0/32/64/96) so the DH=16-contraction score matmuls run 4-way concurrent via
    TensorE row tiling; the A@V matmuls run 4-way concurrent via column tiling.
"""

import numpy as np

import concourse.bass as bass
import concourse.mybir as mybir
import concourse.tile as tile
from concourse.bass_utils import run_bass_kernel_spmd
from concourse.vector_clock import ScopedClock
from contextlib import ExitStack

dt = mybir.dt
AF = mybir.ActivationFunctionType
ALU = mybir.AluOpType

EMB, NH, DHD, FF = 128, 8, 16, 512
B, T, Q = 16, 512, 128
NCORES = 8
BL = B // NCORES  # items per core
NLAYER = 4        # 3 encoder + 1 decoder
MHA_NORM = 1.0 / np.sqrt(DHD)
PTR_NORM = 1.0 / np.sqrt(EMB)
LN_EPS = 1e-5
MASK_BIG = 30000.0

USE_FP32R = False
MMDT = dt.float32r   # matmul-operand dtype
AVDT = dt.bfloat16   # attention A/V dtype


def _install_tile_patch():
    """walrus codegen rejects >2 sync waits on one CTRL instruction; split the
    Tile exit drain's waits across multiple drains."""
    if getattr(tile.TileContext, "_drain_patched", False):
        return

    def _patched(self, tick_clock, wait_clock):
        drain_inst = self.nc.sync.drain()
        wait_clock.add_sem_waits(
            drain_inst.ins, ScopedClock({None: tick_clock.global_clock})
        )
        si = drain_inst.ins.sync_info
        if si is not None and si.on_wait and len(si.on_wait) > 1:
            waits = list(si.on_wait)
            si.on_wait = waits[:1]
            for k in range(1, len(waits)):
                extra = self.nc.sync.drain()
                esi = extra.ins.sync_info
                if esi is None:
                    extra.ins.sync_info = mybir.SyncInfo(
                        on_wait=[waits[k]], on_update=[]
                    )
                else:
                    esi.on_wait = [waits[k]]
        self.nc.all_engine_barrier()
        assert self.sems is not None
        popped = self.nc._tile_sem_poison_stack.pop()
        assert popped is self._sem_poison
        self.nc.clear_and_free_semaphores(list(self.sems.allocated().values()))
        self.nc.all_engine_barrier()

    tile.TileContext._drain_and_barrier = _patched
    tile.TileContext._drain_patched = True


def _mm(ap):
    return ap


def _split_excess_waits(nc, max_waits=1):
    """walrus codegen supports a very small number of sync waits per
    instruction; move excess waits onto freshly inserted same-engine nops."""
    Op = nc.isa.Opcode
    sp_engines = {mybir.EngineType.SP}
    for f in nc.m.functions:
        for b in f.blocks:
            il = b.instructions
            i = 0
            while i < len(il):
                inst = il[i]
                si = inst.sync_info
                if si is not None and si.on_wait and len(si.on_wait) > max_waits:
                    waits = list(si.on_wait)
                    si.on_wait = waits[:max_waits]
                    carriers = []
                    for k, w in enumerate(waits[max_waits:]):
                        raw = mybir.InstEventSemaphore(
                            name=f"waitsplit_{inst.name}_{k}", ins=[], outs=[]
                        )
                        raw.engine = inst.engine
                        raw.sync_info = mybir.SyncInfo(on_wait=[w], on_update=[])
                        carriers.append(raw)
                    il[i:i] = carriers
                    i += len(carriers)
                i += 1


def build_program(split_waits=True):
    _install_tile_patch()
    nc = bass.Bass("TRN2", target_bir_lowering=False, debug=False, num_devices=1)

    def din(name, shape, ddt=dt.float32):
        return nc.dram_tensor(name, shape, ddt, kind="ExternalInput").ap()

    srcT = din("srcT", (BL, EMB, T), MMDT)
    tgtT = din("tgtT", (BL, EMB, Q), MMDT)
    dmaskT = din("dmaskT", (BL, T, Q))      # -MASK_BIG * mask, transposed
    pinv10 = din("pinv10", (BL, EMB, T))    # 10*(ptr_mask==0) row-broadcast
    pneg = din("pneg", (BL, EMB, T))        # -10000*(ptr_mask!=0) row-broadcast
    wqe_d = din("wqe", (EMB, NLAYER * 128), MMDT)
    wqo_d = din("wqo", (EMB, NLAYER * 128), MMDT)
    wke_d = din("wke", (EMB, NLAYER * 128), MMDT)
    wko_d = din("wko", (EMB, NLAYER * 128), MMDT)
    wv_d = din("wv", (EMB, NLAYER * 128), MMDT)
    woe_d = din("woe", (EMB, NLAYER * 128), MMDT)
    woo_d = din("woo", (EMB, NLAYER * 128), MMDT)
    w1_d = din("w1", (EMB, NLAYER * FF), MMDT)
    w2_d = din("w2", (128, NLAYER * 4 * 128), MMDT)
    b1_d = din("b1", (128, NLAYER * 4))
    b2_d = din("b2", (128, NLAYER))
    emat_d = din("emat", (8, 2, 128), MMDT)
    onesc_d = din("onesc", (128, 1), MMDT)
    onesr_d = din("onesr", (1, 128), MMDT)
    pwq_d = din("pwq", (EMB, EMB), MMDT)
    pwk_d = din("pwk", (EMB, EMB), MMDT)
    out_d = nc.dram_tensor("out", (BL, Q, T), dt.float32, kind="ExternalOutput").ap()

    with tile.TileContext(nc) as tc:
        es = ExitStack()
        consts = es.enter_context(tc.tile_pool(name="consts", bufs=1))
        big = es.enter_context(tc.tile_pool(name="big", bufs=2))
        act = es.enter_context(tc.tile_pool(name="act", bufs=2))
        act1 = es.enter_context(tc.tile_pool(name="act1", bufs=1))
        vpool = es.enter_context(tc.tile_pool(name="vpool", bufs=2))
        psub = es.enter_context(tc.tile_pool(name="psub", bufs=2, space="PSUM"))
        pshu = es.enter_context(tc.tile_pool(name="pshu", bufs=2, space="PSUM"))
        psm = es.enter_context(tc.tile_pool(name="psm", bufs=2, space="PSUM"))

        # ---- persistent weights in SBUF
        def wload(name, dram, cols, wdt=MMDT):
            t = consts.tile([128, cols], wdt, tag=name)
            nc.sync.dma_start(t[:], dram[:])
            return t

        wqe = wload("wqe", wqe_d, NLAYER * 128)
        wqo = wload("wqo", wqo_d, NLAYER * 128)
        wke = wload("wke", wke_d, NLAYER * 128)
        wko = wload("wko", wko_d, NLAYER * 128)
        wv = wload("wv", wv_d, NLAYER * 128)
        woe = wload("woe", woe_d, NLAYER * 128)
        woo = wload("woo", woo_d, NLAYER * 128)
        w1 = wload("w1", w1_d, NLAYER * FF)
        w2 = wload("w2", w2_d, NLAYER * 4 * 128)
        b1 = wload("b1", b1_d, NLAYER * 4, dt.float32)
        b2 = wload("b2", b2_d, NLAYER, dt.float32)
        pwq = wload("pwq", pwq_d, EMB)
        pwk = wload("pwk", pwk_d, EMB)

        ones_col = consts.tile([128, 1], MMDT, tag="ones_col")
        nc.sync.dma_start(ones_col[:], onesc_d[:])
        ones_row = consts.tile([1, 128], MMDT, tag="ones_row")
        nc.sync.dma_start(ones_row[:], onesr_d[:])
        emat8 = consts.tile([8, 2, 128], MMDT, tag="emat8")
        nc.sync.dma_start(emat8[:], emat_d[:])
        epsb = consts.tile([1, 1], dt.float32, tag="epsb")
        nc.vector.memset(epsb[:], LN_EPS)

        # expUT buffer: [128, chunk(4), slot(8), 512]
        exput = None  # allocated per item-layer from big pool

        def layer_norm(x, Tn, tagp):
            """x: [128, Tn] SBUF -> h: [128, Tn] SBUF (identity gamma/beta)."""
            x2 = act.tile([128, Tn], MMDT, tag="ln_x2")
            nc.vector.tensor_mul(x2[:], x[:], x[:])
            s1 = psm.tile([1, Tn], dt.float32, tag="pbank")
            s2 = psm.tile([1, Tn], dt.float32, tag="pbank")
            nc.tensor.matmul(s1[:], _mm(ones_col[:]), _mm(x[:]), start=True, stop=True)
            nc.tensor.matmul(s2[:], _mm(ones_col[:]), _mm(x2[:]), start=True, stop=True)
            negmu = act1.tile([1, Tn], dt.float32, tag="ln_negmu")
            nc.vector.tensor_scalar_mul(negmu[:], s1[:], -1.0 / 128)
            mu2 = act1.tile([1, Tn], dt.float32, tag="ln_mu2")
            nc.vector.tensor_mul(mu2[:], negmu[:], negmu[:])
            varr = act1.tile([1, Tn], dt.float32, tag="ln_varr")
            nc.vector.scalar_tensor_tensor(
                out=varr[:], in0=s2[:], scalar=1.0 / 128, in1=mu2[:],
                op0=ALU.mult, op1=ALU.subtract,
            )
            lnv = varr
            nc.scalar.activation(lnv[:], varr[:], AF.Ln, bias=epsb[0:1, :])
            rstd = act1.tile([1, Tn], MMDT, tag="ln_rstd")
            nc.scalar.activation(rstd[:], lnv[:], AF.Exp, scale=-0.5)
            nmr = act1.tile([1, Tn], MMDT, tag="ln_nmr")
            nc.vector.tensor_mul(nmr[:], negmu[:], rstd[:])
            d1 = psm.tile([128, Tn], dt.float32, tag="pbank")
            d2 = psm.tile([128, Tn], dt.float32, tag="pbank")
            nc.tensor.matmul(d1[:], _mm(ones_row[:]), _mm(rstd[:]), start=True, stop=True)
            nc.tensor.matmul(d2[:], _mm(ones_row[:]), _mm(nmr[:]), start=True, stop=True)
            h = act.tile([128, Tn], MMDT, tag="ln_h")
            nc.vector.tensor_mul(h[:], x[:], d1[:])
            nc.vector.tensor_add(h[:], h[:], d2[:])
            return h

        def v_blocks(h, L, Tn):
            """V blocks [128, c(4), g(2), r(4), 32]: col0=ones, 1:17=V head, rest 0."""
            nck = Tn // 128
            vblk = vpool.tile([128, 4, 2, 4, 32], AVDT, tag="vblk")
            nc.vector.memset(vblk[:, :, :, :, 0:1], 1.0)
            nc.vector.memset(vblk[:, :, :, :, 17:32], 0.0)
            for c in range(nck):
                pv = psm.tile([128, 128], dt.float32, tag="pbank")
                nc.tensor.matmul(
                    pv[:], _mm(h[:, 128 * c : 128 * (c + 1)]),
                    _mm(wv[:, 128 * L : 128 * (L + 1)]), start=True, stop=True,
                )
                pv8 = pv.rearrange("p (h k) -> p h k", h=8)
                nc.vector.tensor_copy(vblk[:, c, 0, :, 1:17], pv8[:, 0::2, :])
                nc.vector.tensor_copy(vblk[:, c, 1, :, 1:17], pv8[:, 1::2, :])
            return vblk

        def attention(qte, qto, kte, kto, vblk, L, Tq, dmask_sb):
            """Returns Hn_e, Hn_o (normalized per-head values, block layout)."""
            qt_g = (qte, qto)
            kt_g = (kte, kto)
            ex4 = big.tile([128, 4, 8, T], AVDT, tag="exput")
            # --- scores + exp
            for c in range(4):
                ui_c = None
                for g in range(2):
                    for half in range(2):
                        # one matmul per PSUM bank (HW-validated config)
                        ub = psub.tile([128, 2 * T], dt.float32, tag="ub")
                        for rr in range(2):
                            r = 2 * half + rr
                            nc.tensor.matmul(
                                ub[:, rr * T : rr * T + Tq],
                                _mm(kt_g[g][32 * r : 32 * r + DHD, 128 * c : 128 * (c + 1)]),
                                _mm(qt_g[g][32 * r : 32 * r + DHD, :Tq]),
                                start=True, stop=True, tile_position=(32 * r, 0),
                            )
                        s0 = 4 * g + 2 * half
                        ubv = ub.rearrange("p (a b) -> p a b", a=2)[:, :, :Tq]
                        if dmask_sb is None:
                            nc.scalar.activation(
                                ex4[:, c, s0 : s0 + 2, :Tq], ubv,
                                AF.Exp, scale=MHA_NORM,
                            )
                        else:
                            # stage masked scores for all 8 slots, one exp per chunk
                            if ui_c is None:
                                ui_c = act1.tile([128, 8, Tq], dt.float32, tag="dec_ui", bufs=2)
                            mrow = dmask_sb[:, 128 * c : 128 * (c + 1)].unsqueeze(1)
                            nc.vector.scalar_tensor_tensor(
                                out=ui_c[:, s0 : s0 + 2, :], in0=ubv,
                                scalar=MHA_NORM, in1=mrow.to_broadcast((128, 2, Tq)),
                                op0=ALU.mult, op1=ALU.add,
                            )
                if dmask_sb is not None:
                    nc.scalar.activation(
                        ex4[:, c, :, :Tq], ui_c[:], AF.Exp,
                    )
            # --- A@V with aug-ones (column tiling), then normalize
            husbs = []
            sg = act1.tile([8, Tq], dt.float32, tag="sgat", bufs=2)
            for g in range(2):
                hu = pshu.tile([128, Tq], dt.float32, tag="hu")
                for c in range(4):
                    for r in range(4):
                        nc.tensor.matmul(
                            hu[32 * r : 32 * r + 32, :],
                            _mm(vblk[:, c, g, r, :]),
                            _mm(ex4[:, c, 4 * g + r, :Tq]),
                            start=(c == 0), stop=(c == 3),
                            tile_position=(0, 32 * r), skip_group_check=True,
                        )
                husb = act.tile([128, Tq], dt.float32, tag="husb")
                nc.vector.tensor_copy(husb[:], hu[:])
                husb_r = husb.rearrange("(h a) t -> h a t", a=32)
                nc.sync.dma_start(sg[4 * g : 4 * g + 4, :], husb_r[:, 0, :])
                husbs.append(husb)
            # rec = 1/s via exp(-ln(s)) (ScalarE; keeps everything in one table set)
            lns = act1.tile([8, Tq], dt.float32, tag="lns", bufs=2)
            nc.scalar.activation(lns[:], sg[:], AF.Ln)
            rec = act1.tile([8, Tq], MMDT, tag="rec", bufs=2)
            nc.scalar.activation(rec[:], lns[:], AF.Exp, scale=-1.0)
            hns = []
            for g in range(2):
                dmat = psm.tile([128, Tq], dt.float32, tag="pbank")
                nc.tensor.matmul(
                    dmat[:], _mm(emat8[:, g, :]), _mm(rec[:]), start=True, stop=True
                )
                hn = act.tile([128, Tq], MMDT, tag="hn")
                nc.vector.tensor_mul(hn[:], husbs[g][:], dmat[:])
                hns.append(hn)
            return hns

        def ffn_block(x2t, L, Tn, tag_out="x"):
            h2 = layer_norm(x2t, Tn, "ln2")
            h1 = act.tile([128, 4, Tn], MMDT, tag="h1", bufs=1)
            for cx in range(4):
                pf = psm.tile([128, Tn], dt.float32, tag="pbank")
                nc.tensor.matmul(
                    pf[:], _mm(w1[:, FF * L + 128 * cx : FF * L + 128 * (cx + 1)]),
                    _mm(h2[:]), start=True, stop=True,
                )
                nc.vector.tensor_scalar(
                    out=h1[:, cx, :], in0=pf[:],
                    scalar1=b1[:, 4 * L + cx : 4 * L + cx + 1], op0=ALU.add,
                    scalar2=0.0, op1=ALU.max,
                )
            po = psm.tile([128, Tn], dt.float32, tag="pbank")
            for cx in range(4):
                nc.tensor.matmul(
                    po[:], _mm(w2[:, (4 * L + cx) * 128 : (4 * L + cx + 1) * 128]),
                    _mm(h1[:, cx, :]), start=(cx == 0), stop=(cx == 3),
                )
            xo = act.tile([128, Tn], MMDT, tag=tag_out)
            nc.vector.scalar_tensor_tensor(
                out=xo[:], in0=po[:], scalar=b2[:, L : L + 1], in1=x2t[:],
                op0=ALU.add, op1=ALU.add,
            )
            return xo

        def enc_dec_layer(x, L, Tq, kv_src, dmask_sb, tag_out="x"):
            """One transformer layer. kv_src is None for encoder (self-attn)."""
            hq = layer_norm(x, Tq, "ln1")
            hkv = hq if kv_src is None else layer_norm(kv_src, T, "ln1m")

            def proj(wt, rhs, Tn, nm):
                p = psm.tile([128, Tn], dt.float32, tag="pbank")
                nc.tensor.matmul(
                    p[:], _mm(wt[:, 128 * L : 128 * (L + 1)]), _mm(rhs[:]),
                    start=True, stop=True,
                )
                o = act.tile([128, Tn], MMDT, tag=f"qk_{nm}")
                nc.vector.tensor_copy(o[:], p[:])
                return o

            qte = proj(wqe, hq, Tq, "qte")
            qto = proj(wqo, hq, Tq, "qto")
            kte = proj(wke, hkv, T, "kte")
            kto = proj(wko, hkv, T, "kto")
            vblk = v_blocks(hkv, L, T)
            hn_e, hn_o = attention(qte, qto, kte, kto, vblk, L, Tq, dmask_sb)
            pa = psm.tile([128, Tq], dt.float32, tag="pbank")
            nc.tensor.matmul(pa[:], _mm(woe[:, 128 * L : 128 * (L + 1)]), _mm(hn_e[:]), start=True, stop=False)
            nc.tensor.matmul(pa[:], _mm(woo[:, 128 * L : 128 * (L + 1)]), _mm(hn_o[:]), start=False, stop=True)
            x2t = act.tile([128, Tq], MMDT, tag="x2t")
            nc.vector.tensor_add(x2t[:], pa[:], x[:])
            return ffn_block(x2t, L, Tq, tag_out)

        # ================= item-interleaved pipeline =================
        xs = []
        for b in range(BL):
            x = act.tile([128, T], MMDT, tag="x")
            nc.sync.dma_start(x[:], srcT[b])
            xs.append(x)
        for L in range(3):
            for b in range(BL):
                tag_out = "mem" if L == 2 else "x"
                xs[b] = enc_dec_layer(xs[b], L, T, None, None, tag_out)
        mems = xs

        decs = []
        for b in range(BL):
            tg = act.tile([128, Q], MMDT, tag="tg")
            nc.sync.dma_start(tg[:], tgtT[b])
            dmask_sb = act1.tile([128, T], dt.float32, tag="dmask", bufs=2)
            dm_r = dmaskT[b].rearrange("(c p) q -> p c q", p=128)
            nc.sync.dma_start(dmask_sb.rearrange("p (c q) -> p c q", c=4), dm_r)
            decs.append(enc_dec_layer(tg, 3, Q, mems[b], dmask_sb, "x"))

        for b in range(BL):
            out_dec, mem = decs[b], mems[b]
            # ---- pointer head
            pq = psm.tile([128, Q], dt.float32, tag="pbank")
            nc.tensor.matmul(pq[:], _mm(pwq[:]), _mm(out_dec[:]), start=True, stop=True)
            qp = act1.tile([128, Q], MMDT, tag="qp")
            nc.vector.tensor_copy(qp[:], pq[:])
            pk = psm.tile([128, T], dt.float32, tag="pbank")
            nc.tensor.matmul(pk[:], _mm(pwk[:]), _mm(mem[:]), start=True, stop=True)
            kp = act1.tile([128, T], MMDT, tag="kp")
            nc.vector.tensor_copy(kp[:], pk[:])
            pu = psm.tile([128, T], dt.float32, tag="pbank")
            nc.tensor.matmul(pu[:], _mm(qp[:]), _mm(kp[:]), start=True, stop=True)
            # 10*tanh(z) = 10*(e2z-1)/(e2z+1),  z = PTR_NORM * U
            e2z = act1.tile([128, T], dt.float32, tag="e2z")
            nc.scalar.activation(e2z[:], pu[:], AF.Exp, scale=2.0 * PTR_NORM)
            den = act1.tile([128, T], dt.float32, tag="den")
            nc.vector.tensor_scalar_add(den[:], e2z[:], 1.0)
            nc.scalar.activation(den[:], den[:], AF.Ln)
            nc.scalar.activation(den[:], den[:], AF.Exp, scale=-1.0)  # rden
            num = act1.tile([128, T], dt.float32, tag="num")
            nc.vector.tensor_scalar(
                out=num[:], in0=e2z[:], scalar1=10.0, op0=ALU.mult,
                scalar2=10.0, op1=ALU.subtract,
            )
            nc.vector.tensor_mul(num[:], num[:], den[:])  # 10*tanh
            pin = act1.tile([128, T], dt.float32, tag="pin")
            nc.sync.dma_start(pin[:], pinv10[b])
            png = act1.tile([128, T], dt.float32, tag="png")
            nc.sync.dma_start(png[:], pneg[b])
            uq = num  # U'' built in place
            nc.vector.tensor_mul(uq[:], uq[:], pin[:])
            nc.vector.tensor_add(uq[:], uq[:], png[:])
            # log-softmax over free dim
            rmax = act1.tile([128, 1], dt.float32, tag="rmax")
            nc.vector.tensor_reduce(rmax[:], uq[:], mybir.AxisListType.X, ALU.max)
            nrmax = act1.tile([128, 1], dt.float32, tag="nrmax")
            nc.vector.tensor_scalar_mul(nrmax[:], rmax[:], -1.0)
            ex = act1.tile([128, T], dt.float32, tag="ex")
            rsum = act1.tile([128, 1], dt.float32, tag="rsum")
            nc.scalar.activation(ex[:], uq[:], AF.Exp, bias=nrmax[:], accum_out=rsum[:])
            lse = act1.tile([128, 1], dt.float32, tag="lse")
            nc.scalar.activation(lse[:], rsum[:], AF.Ln)
            shift = act1.tile([128, 1], dt.float32, tag="shift")
            nc.vector.tensor_add(shift[:], rmax[:], lse[:])
            res = ex
            nc.vector.tensor_scalar(
                out=res[:], in0=uq[:], scalar1=shift[:], scalar2=None,
                op0=ALU.subtract,
            )
            nc.sync.dma_start(out_d[b], res[:])
        es.close()
    return nc


def prep_inputs(src, tgt, enc_params, dec_params, ptr_wq, ptr_wk, dec_mask, ptr_mask):
    """Host-side layout prep: shard over cores, transpose activations, pack weights."""
    src = np.asarray(src, np.float32)
    tgt = np.asarray(tgt, np.float32)
    dec_mask = np.asarray(dec_mask)
    ptr_mask = np.asarray(ptr_mask)

    layers = list(enc_params) + list(dec_params)
    assert len(layers) == NLAYER

    def packW(key, even):
        # heads 2g(+1) columns -> cols 32g..32g+16 of a [128,128] block
        out = np.zeros((EMB, NLAYER * 128), np.float32)
        for L, p in enumerate(layers):
            w = np.asarray(p[key], np.float32)  # [NH, EMB, DHD]
            for g in range(4):
                h = 2 * g + (0 if even else 1)
                out[:, 128 * L + 32 * g : 128 * L + 32 * g + DHD] = w[h]
        return out

    wqe, wqo = packW("wq", True), packW("wq", False)
    wke, wko = packW("wk", True), packW("wk", False)

    wv = np.zeros((EMB, NLAYER * 128), np.float32)
    for L, p in enumerate(layers):
        w = np.asarray(p["wv"], np.float32)  # [NH, EMB, DHD]
        wv[:, 128 * L : 128 * (L + 1)] = w.transpose(1, 0, 2).reshape(EMB, 128)

    def packWo(even):
        # lhsT rows: block r: row 32r = 0 (sums), rows 32r+1..+17 = wout[h] rows
        out = np.zeros((EMB, NLAYER * 128), np.float32)
        for L, p in enumerate(layers):
            w = np.asarray(p["wout"], np.float32)  # [NH, DHD, EMB]
            for r in range(4):
                h = 2 * r + (0 if even else 1)
                out[32 * r + 1 : 32 * r + 1 + DHD, 128 * L : 128 * (L + 1)] = w[h]
        return out

    woe, woo = packWo(True), packWo(False)

    w1 = np.concatenate([np.asarray(p["w1"], np.float32) for p in layers], axis=1)
    w2 = np.zeros((128, NLAYER * 4 * 128), np.float32)
    b1 = np.zeros((128, NLAYER * 4), np.float32)
    b2 = np.zeros((128, NLAYER), np.float32)
    for L, p in enumerate(layers):
        w2f = np.asarray(p["w2"], np.float32)  # [FF, EMB]
        for c in range(4):
            w2[:, (4 * L + c) * 128 : (4 * L + c + 1) * 128] = w2f[128 * c : 128 * (c + 1)]
        b1[:, 4 * L : 4 * (L + 1)] = np.asarray(p["b1"], np.float32).reshape(4, 128).T
        b2[:, L] = np.asarray(p["b2"], np.float32)

    emat = np.zeros((8, 2, 128), np.float32)
    for g in range(2):
        for c in range(4):
            emat[4 * g + c, g, 32 * c : 32 * (c + 1)] = 1.0
    weights = dict(
        wqe=wqe, wqo=wqo, wke=wke, wko=wko, wv=wv, woe=woe, woo=woo,
        w1=np.ascontiguousarray(w1), w2=w2, b1=b1, b2=b2, emat=emat,
        onesc=np.ones((128, 1), np.float32), onesr=np.ones((1, 128), np.float32),
        pwq=np.asarray(ptr_wq, np.float32), pwk=np.asarray(ptr_wk, np.float32),
    )

    in_maps = []
    for core in range(NCORES):
        sl = slice(core * BL, (core + 1) * BL)
        srcT = np.ascontiguousarray(src[sl].transpose(0, 2, 1))
        tgtT = np.ascontiguousarray(tgt[sl].transpose(0, 2, 1))
        dmaskT = np.ascontiguousarray(
            (-MASK_BIG * (dec_mask[sl] != 0)).astype(np.float32).transpose(0, 2, 1)
        )
        pm = ptr_mask[sl]
        pinv10 = np.ascontiguousarray(
            np.broadcast_to((pm == 0).astype(np.float32)[:, None, :], (BL, EMB, T))
        )
        pneg = np.ascontiguousarray(
            np.broadcast_to((-10000.0 * (pm != 0)).astype(np.float32)[:, None, :], (BL, EMB, T))
        )
        in_maps.append(
            dict(srcT=srcT, tgtT=tgtT, dmaskT=dmaskT, pinv10=pinv10, pneg=pneg, **weights)
        )
    return in_maps


_PROGRAM = None


def kernel(src, tgt, enc_params, dec_params, ptr_wq, ptr_wk, dec_mask, ptr_mask):
    global _PROGRAM
    if _PROGRAM is None:
        _PROGRAM = build_program()
    in_maps = prep_inputs(
        src, tgt, enc_params, dec_params, ptr_wq, ptr_wk, dec_mask, ptr_mask
    )
    res = run_bass_kernel_spmd(_PROGRAM, in_maps, list(range(NCORES)))
    return np.concatenate([res.results[i]["out"] for i in range(NCORES)], axis=0)
